# revision 14
# baseline (speedup 1.0000x reference)
"""Trainium2 Bass kernel for nn_Decoder (LSTM(input=1,hidden=512) over S=256 steps + FC).

Data-parallel over batch: B=256 -> 32 rows/core on 8 cores; weights replicated.
Feature-major layout (gate features on SBUF partitions, batch on the free dim);
the recurrent matmul is weight-stationary: 64 [128,128] lhsT tiles per step,
one PSUM bank per gate (order f,g,i,o), one accumulation group per bank.

Perf design (vs. the ~904us bf16 baseline, LDWEIGHTS-byte-bound at ~54ns/tile):
- fp8(e4m3) weights, scaled x64 on the host (clears the subnormal floor), with
  bf16 moving operand: halves LDWEIGHTS bytes -> ~21ns/tile pairs. The 1/64
  descale rides the gate activations' free `scale` parameter.
- LSTM state error from fp8 weights saturates (forget-gate damping), so the
  LAST `tail_k`=16 steps run against a second bf16 copy of the weights, which
  collapses the final-state error to the bf16 noise floor (~2.4e-3).
- The whole rhs sequence ([y_t;1] blocks for the x*W_ih+bias fold-in) is
  preloaded to SBUF once: zero in-loop DMA.
- tanh(c) is emitted right after the c-update (i-bank phase) so it runs on
  ScalarE under the o-bank matmuls; the exposed per-step tail is only
  sigmoid(o)->h-mul, chunked in halves (tail_split=2) so the next step's
  matmuls start after the first 64 columns of h.
"""

import os
import sys

sys.path.insert(0, "/opt/trn_rl_repo")

import ml_dtypes
import numpy as np

import concourse.mybir as mybir
import concourse.tile as tile
from concourse import bacc
from concourse.bass_utils import run_bass_kernel_spmd

B, S, H, OUT = 256, 256, 512, 128
NCORES = 8
BL = B // NCORES  # 32 batch rows per core

bf16 = mybir.dt.bfloat16
f32 = mybir.dt.float32

# gate slot order within a slice's 128 cols: [i | f | o | g]
GS_TG = [0, 1, 3, 2]  # gate slot -> torch gate row-block index (i,f,g,o order)

_BUILD_CACHE: dict = {}


def _feat_index():
    """feat[a, gs, p] = row index in W_hh/b for slice a, gate slot gs, partition p."""
    a = np.arange(4)[:, None, None]
    gs = np.array(GS_TG)[None, :, None]
    p = np.arange(128)[None, None, :]
    return (gs * H + a * 128 + p).astype(np.int64)  # (4,4,128)


def step_body(nc, n_steps, Hbuf, C, W, XW, rhsp, work, psum, rhsd, groups,
              do_mm=True, do_elem=True, split_h=False, split_sig=False):
    AF = mybir.ActivationFunctionType
    A = 4 // groups  # hidden slices per group
    ng = 128 * A  # psum cols per group
    kg = 8 * A  # xb-matmul contraction rows per group
    for t in range(n_steps):
        hprev = Hbuf[t % 2]
        hnext = Hbuf[(t + 1) % 2]
        rts = [
            rhsp.tile([kg, ng], bf16, name=f"rt{g}", tag=f"rt{g}")
            for g in range(groups)
        ]
        for g in range(groups):
            nc.sync.dma_start(
                rts[g][:],
                rhsd[t, g * kg : (g + 1) * kg, g * ng : (g + 1) * ng],
            )
        pbs = [
            psum.tile([128, ng], f32, name=f"pb{g}", tag=f"pb{g}")
            for g in range(groups)
        ]
        for g in range(groups):
            pb = pbs[g]
            # xb matmul first: writes the whole group tile (start=True), so
            # every later matmul accumulates onto set has_written bits and
            # emission order of the disjoint gate regions can't corrupt data.
            nc.tensor.matmul(
                pb[:],
                XW[g][:],
                rts[g][:],
                start=True,
                stop=not do_mm,
                skip_group_check=True,
            )
            # gate matmuls: g-region first so tanh(g) can run under the rest
            if do_mm:
                for gs in (3, 0, 1, 2):
                    for al in range(A):
                        a = g * A + al
                        for k in range(4):
                            w_off = ((a * 4 + gs) * 4 + k) * 128
                            nc.tensor.matmul(
                                pb[:, al * 128 + 32 * gs : al * 128 + 32 * gs + 32],
                                W[:, w_off : w_off + 128],
                                hprev[:, 32 * k : 32 * k + 32],
                                start=False,
                                stop=(gs == 2 and al == A - 1 and k == 3),
                                skip_group_check=True,
                            )
        if not do_elem:
            continue
        for g in range(groups):
            pb3 = pbs[g].rearrange("p (a c) -> p a c", c=128)  # (128, A, 128)
            cg3 = C[:, g * 32 * A : (g + 1) * 32 * A].rearrange(
                "p (a c) -> p a c", c=32
            )  # (128, A, 32)
            Gt = work.tile([128, A, 32], f32, name="tg", tag="tg")
            nc.scalar.activation(Gt[:], pb3[:, :, 96:128], AF.Tanh)
            if split_sig:
                Sf = work.tile([128, A, 32], f32, name="sf", tag="sf")
                nc.scalar.activation(Sf[:], pb3[:, :, 32:64], AF.Sigmoid)
                Sio = work.tile([128, A, 2, 32], f32, name="sio", tag="sio")
                nc.scalar.activation(
                    Sio[:],
                    pb3.rearrange("p a c -> p a c")[:, :, 0:96].rearrange(
                        "p a (i c) -> p a i c", c=32
                    )[:, :, 0::2, :],
                    AF.Sigmoid,
                )
                s_i, s_f, s_o = Sio[:, :, 0, :], Sf[:], Sio[:, :, 1, :]
            else:
                Sg = work.tile([128, A, 96], f32, name="sig", tag="sig")
                nc.scalar.activation(Sg[:], pb3[:, :, 0:96], AF.Sigmoid)
                s_i, s_f, s_o = Sg[:, :, 0:32], Sg[:, :, 32:64], Sg[:, :, 64:96]
            T2 = work.tile([128, A, 32], f32, name="t2", tag="t2")
            nc.vector.tensor_mul(out=T2[:], in0=s_f, in1=cg3)
            T1 = work.tile([128, A, 32], f32, name="t1", tag="t1")
            nc.vector.tensor_mul(out=T1[:], in0=s_i, in1=Gt[:])
            nc.vector.tensor_add(out=cg3, in0=T1[:], in1=T2[:])
            TC = work.tile([128, A, 32], f32, name="tc", tag="tc")
            nc.scalar.activation(TC[:], cg3, AF.Tanh)
            hout = hnext[:, g * 32 * A : (g + 1) * 32 * A].rearrange(
                "p (a c) -> p a c", c=32
            )
            if split_h:
                for al in range(A):
                    nc.vector.tensor_mul(
                        out=hout[:, al : al + 1, :],
                        in0=s_o[:, al : al + 1, :],
                        in1=TC[:, al : al + 1, :],
                    )
            else:
                nc.vector.tensor_mul(out=hout, in0=s_o, in1=TC[:])


def step_body_v3(nc, n_steps, Hbuf, C, W, XW, rhsp, work, psum, rhsd,
                 do_mm=True, do_elem=True, split_h=False, chain_split=1,
                 tail_split=None, k_outer=False, inv_s=1.0, RT=None,
                 Wtail=None, tail_k=0):
    if tail_split is None:
        tail_split = chain_split
    """Bank-per-gate layout: each gate's 4 hidden-slices accumulate in its own
    PSUM bank (order f,g,i,o), so per-gate activations overlap the next gate's
    matmuls on a different bank.  Only sigmoid(o) -> h remains after the MM
    phase.  xb matmuls are emitted one step ahead so they fill the PE bubble
    during the activation tail.

    inv_s: descale factor applied in the gate activations (weights were
    pre-scaled by 1/inv_s on the host, e.g. for fp8 quantization).
    RT: optional preloaded rhs tile [8, n_steps*128]; skips per-step DMA."""
    AF = mybir.ActivationFunctionType

    def alloc_step(t):
        if RT is not None:
            rt_ap = RT[:, t * 128 : (t + 1) * 128]
        else:
            rt = rhsp.tile([8, 128], bf16, name="rt", tag="rt")
            nc.sync.dma_start(rt[:], rhsd[t])
            rt_ap = rt[:]
        pbs = [
            psum.tile([128, 128], f32, name=f"pb{b}", tag=f"pb{b}")
            for b in range(4)
        ]
        for b in range(4):
            nc.tensor.matmul(
                pbs[b][:], XW[b][:], rt_ap, start=True, stop=False,
                skip_group_check=True,
            )
        return pbs

    def hsl(hbuf, k):
        return hbuf[:, 32 * k : 32 * k + 32]

    pbs_cur = alloc_step(0)
    import os as _os
    fake_h = bool(int(_os.environ.get("LSTM_FAKE_H", "0")))
    for t in range(n_steps):
        Wt = Wtail if (Wtail is not None and t >= n_steps - tail_k) else W
        hprev = Hbuf[0] if fake_h else Hbuf[t % 2]
        hnext = Hbuf[(t + 1) % 2]
        pbs = pbs_cur
        acts = {}
        for b in range(4):  # bank order f, g, i, o
            pb = pbs[b]
            if do_mm:
                if k_outer == 3:  # al-major within k-halves
                    mm_iter = ([(al, k) for al in range(4) for k in (0, 1)]
                               + [(al, k) for al in range(4) for k in (2, 3)])
                elif k_outer == 2:  # k-pair-major: matches 64-col hnext chunks
                    mm_iter = ([(al, k) for k in (0, 1) for al in range(4)]
                               + [(al, k) for k in (2, 3) for al in range(4)])
                elif k_outer:
                    mm_iter = [(al, k) for k in range(4) for al in range(4)]
                else:
                    mm_iter = [(al, k) for al in range(4) for k in range(4)]
                for idx, (al, k) in enumerate(mm_iter):
                    w_off = ((b * 4 + al) * 4 + k) * 128
                    nc.tensor.matmul(
                        pb[:, al * 32 : al * 32 + 32],
                        Wt[:, w_off : w_off + 128],
                        hsl(hprev, k),
                        start=False,
                        stop=(idx == 15),
                        skip_group_check=True,
                    )
            if not do_elem:
                continue
            if b == 0:  # f
                Sf = work.tile([128, 128], f32, name="sf", tag="sf")
                nc.scalar.activation(Sf[:], pb[:], AF.Sigmoid, scale=inv_s)
                T2 = work.tile([128, 128], f32, name="t2", tag="t2")
                nc.vector.tensor_mul(out=T2[:], in0=Sf[:], in1=C[:])
                acts["T2"] = T2
            elif b == 1:  # g
                Gt = work.tile([128, 128], f32, name="tg", tag="tg")
                nc.scalar.activation(Gt[:], pb[:], AF.Tanh, scale=inv_s)
                acts["G"] = Gt
            elif b == 2:  # i
                cs = chain_split
                w_ = 128 // cs
                Si = work.tile([128, 128], f32, name="si", tag="si")
                T1 = work.tile([128, 128], f32, name="t1", tag="t1")
                for hh in range(cs):
                    sl_ = slice(hh * w_, (hh + 1) * w_)
                    nc.scalar.activation(Si[:, sl_], pb[:, sl_], AF.Sigmoid,
                                         scale=inv_s)
                for hh in range(cs):
                    sl_ = slice(hh * w_, (hh + 1) * w_)
                    nc.vector.tensor_mul(
                        out=T1[:, sl_], in0=Si[:, sl_], in1=acts["G"][:, sl_]
                    )
                    nc.vector.tensor_add(
                        out=C[:, sl_], in0=T1[:, sl_], in1=acts["T2"][:, sl_]
                    )
                # tanh(c) here: C is final, so these ACT ops run while the
                # o-bank matmuls occupy the PE, taking tanh off the tail path
                TC = work.tile([128, 128], f32, name="tc", tag="tc")
                for hh in range(cs):
                    sl_ = slice(hh * w_, (hh + 1) * w_)
                    nc.scalar.activation(TC[:, sl_], C[:, sl_], AF.Tanh)
                acts["TC"] = TC
            else:  # o
                # next step's xb matmuls first: they are dependency-free, so
                # the PE can run them during this step's activation tail
                if t + 1 < n_steps:
                    pbs_cur = alloc_step(t + 1)
                So = work.tile([128, 128], f32, name="so", tag="so")
                TC = acts["TC"]
                ts_ = tail_split
                wt = 128 // ts_
                for hh in range(ts_):
                    sl_ = slice(hh * wt, (hh + 1) * wt)
                    nc.scalar.activation(So[:, sl_], pb[:, sl_], AF.Sigmoid,
                                         scale=inv_s)
                    nc.vector.tensor_mul(
                        out=hnext[:, sl_], in0=So[:, sl_], in1=TC[:, sl_]
                    )
        if not do_elem and t + 1 < n_steps:
            pbs_cur = alloc_step(t + 1)


def build_kernel(n_steps: int, outer: int = 1, groups: int = 1,
                 do_mm=True, do_elem=True, split_h=False, split_sig=False,
                 by_gate=False, chain_split=1, tail_split=None, k_outer=False,
                 fp8=False, rhs_pre=False, scale=64.0, tail_k=16):
    """outer>1 wraps the recurrence in a device-side repeat loop (bench only)."""
    wdt = mybir.dt.float8e4 if fp8 else bf16
    nc = bacc.Bacc(None)
    wsb = nc.declare_dram_parameter("wsb", [128, 64 * 128], wdt, isOutput=False)
    if fp8 and tail_k:
        wsb2 = nc.declare_dram_parameter("wsb2", [128, 64 * 128], bf16,
                                         isOutput=False)
    xw = nc.declare_dram_parameter("xw", [32, 128], bf16, isOutput=False)
    if rhs_pre:
        assert by_gate
        rhs_shape = [8, n_steps * 128]
    else:
        rhs_shape = [n_steps, 8, 128] if by_gate else [n_steps, 32, 512]
    rhsd = nc.declare_dram_parameter("rhs", rhs_shape, bf16, isOutput=False)
    h0t = nc.declare_dram_parameter("h0t", [128, 128], bf16, isOutput=False)
    c0t = nc.declare_dram_parameter("c0t", [128, 128], f32, isOutput=False)
    wfc = nc.declare_dram_parameter("wfc", [128, 512], bf16, isOutput=False)
    bfc = nc.declare_dram_parameter("bfc", [128, 1], f32, isOutput=False)
    outt = nc.declare_dram_parameter("outt", [128, BL], f32, isOutput=True)

    with tile.TileContext(nc) as tc:
        with (
            tc.tile_pool(name="const", bufs=1) as constp,
            tc.tile_pool(name="rhsp", bufs=6) as rhsp,
            tc.tile_pool(name="work", bufs=3) as work,
            tc.tile_pool(name="psum", bufs=2, space="PSUM") as psum,
        ):
            W = constp.tile([128, 64 * 128], wdt, tag="W")
            nc.sync.dma_start(W[:], wsb[:])
            Wtail = None
            if fp8 and tail_k:
                Wtail = constp.tile([128, 64 * 128], bf16, tag="Wtail")
                nc.sync.dma_start(Wtail[:], wsb2[:])
            kg_ = 8 if by_gate else 8 * (4 // groups)
            n_xw = 4 if by_gate else groups
            XW = [
                constp.tile([kg_, 128], bf16, name=f"XW{g}", tag=f"XW{g}")
                for g in range(n_xw)
            ]
            RT = None
            if rhs_pre:
                RT = constp.tile([8, n_steps * 128], bf16, tag="RT")
                nc.sync.dma_start(RT[:], rhsd[:])
            for g in range(n_xw):
                nc.sync.dma_start(XW[g][:], xw[g * kg_ : (g + 1) * kg_, :])
            WF = constp.tile([128, 512], bf16, tag="WF")
            nc.sync.dma_start(WF[:], wfc[:])
            BF = constp.tile([128, 1], f32, tag="BF")
            nc.sync.dma_start(BF[:], bfc[:])
            C = constp.tile([128, 128], f32, tag="C")
            nc.sync.dma_start(C[:], c0t[:])
            Hbuf = [constp.tile([128, 128], bf16, name=f"H{p}", tag=f"H{p}")
                    for p in range(2)]
            for p in range(1 if do_elem else 2):
                nc.sync.dma_start(Hbuf[p][:], h0t[:])

            import contextlib

            loop_cm = tc.For_i(0, outer, 1) if outer > 1 else contextlib.nullcontext()
            with loop_cm:
                if by_gate:
                    step_body_v3(nc, n_steps, Hbuf, C, W, XW, rhsp, work, psum,
                                 rhsd, do_mm=do_mm, do_elem=do_elem,
                                 split_h=split_h, chain_split=chain_split,
                                 tail_split=tail_split, k_outer=k_outer,
                                 inv_s=(1.0 / scale) if fp8 else 1.0, RT=RT,
                                 Wtail=Wtail, tail_k=tail_k)
                else:
                    step_body(nc, n_steps, Hbuf, C, W, XW, rhsp, work, psum,
                              rhsd, groups, do_mm=do_mm, do_elem=do_elem,
                              split_h=split_h, split_sig=split_sig)

            # final FC: outT(128, 32) = W_fc @ h_final (+ b_fc)
            hfin = Hbuf[n_steps % 2]
            pfc = psum.tile([128, 128], f32, name="pfc", tag="pb0")
            for k in range(4):
                nc.tensor.matmul(
                    pfc[:, 0:32],
                    WF[:, 128 * k : 128 * k + 128],
                    hfin[:, 32 * k : 32 * k + 32],
                    start=(k == 0),
                    stop=(k == 3),
                )
            osb = work.tile([128, BL], f32, name="osb", tag="osb")
            nc.vector.tensor_scalar_add(osb[:], pfc[:, 0:32], BF[:])
            nc.sync.dma_start(outt[:], osb[:])

    nc.finalize()
    return nc


def _prep_core_inputs(core, y_hist, h0, c0, W_ih, W_hh, bias, W_fc, b_fc, n_steps,
                      by_gate=False, fp8=False, rhs_pre=False, scale=64.0):
    sl = slice(core * BL, (core + 1) * BL)
    y_c = y_hist[sl]  # (32, S)

    if by_gate:
        # feat[b, al, p]: bank order f,g,i,o over torch row-blocks i,f,g,o
        bg = np.array([1, 2, 0, 3])[:, None, None]
        al = np.arange(4)[None, :, None]
        p = np.arange(128)[None, None, :]
        feat = (bg * H + al * 128 + p).astype(np.int64)  # (4,4,128)
        # wsb[r, ((b*4+al)*4+k)*128 + m] = W_hh[feat[b,al,m], k*128+r]
        t = W_hh[feat].reshape(4, 4, 128, 4, 128)  # (b,al,m,k,r)
        wsb = np.ascontiguousarray(t.transpose(4, 0, 1, 3, 2).reshape(128, 64 * 128))
        # xw[b*8+al*2+j, p]
        wih_f = W_ih[:, 0][feat]
        b_f = bias[feat]
        xw = np.stack([wih_f, b_f], axis=2).reshape(32, 128)
        # rhs[t, al*2+j, al*32+n]
        rhs = np.zeros((n_steps, 8, 128), np.float32)
        for a in range(4):
            cols = slice(a * 32, a * 32 + 32)
            rhs[:, a * 2 + 0, cols] = y_c.T[:n_steps]
            rhs[:, a * 2 + 1, cols] = 1.0
    else:
        feat = _feat_index()  # (4,4,128)
        # wsb[r, ((a*4+gs)*4+k)*128 + m] = W_hh[feat[a,gs,m], k*128+r]
        t = W_hh[feat]  # (4,4,128m,512)
        t = t.reshape(4, 4, 128, 4, 128)  # (a,gs,m,k,r)
        wsb = np.ascontiguousarray(t.transpose(4, 0, 1, 3, 2).reshape(128, 64 * 128))
        # xw[a*8+s*2+j, p] = W_ih[feat[a,s,p]] (j=0) or bias[feat[a,s,p]] (j=1)
        wih_f = W_ih[:, 0][feat]  # (4,4,128)
        b_f = bias[feat]  # (4,4,128)
        xw = np.stack([wih_f, b_f], axis=2).reshape(32, 128)  # (a,s,j,p)->(32,128)
        # rhs[t, a*8+s*2+j, a*128+s*32+n] = y[n,t] (j=0) or 1.0 (j=1)
        rhs = np.zeros((n_steps, 32, 512), np.float32)
        for a in range(4):
            for s in range(4):
                cols = slice(a * 128 + s * 32, a * 128 + s * 32 + 32)
                rhs[:, a * 8 + s * 2 + 0, cols] = y_c.T[:n_steps]
                rhs[:, a * 8 + s * 2 + 1, cols] = 1.0

    def t128(x):  # (32, 512) -> (128, 128): out[p, 32k+n] = x[n, k*128+p]
        return np.ascontiguousarray(
            x.T.reshape(4, 128, BL).transpose(1, 0, 2).reshape(128, 4 * BL)
        )

    h0t = t128(h0[sl])
    c0t = t128(c0[sl])

    # wfc[p, k*128+m] = W_fc[m, k*128+p]
    wfc = np.ascontiguousarray(
        W_fc.reshape(OUT, 4, 128).transpose(2, 1, 0).reshape(128, 512)
    )

    extra = {}
    if fp8:
        wdt_np = ml_dtypes.float8_e4m3
        wsb = wsb * scale
        xw = xw * scale
        extra["wsb2"] = wsb.astype(ml_dtypes.bfloat16)
    else:
        wdt_np = ml_dtypes.bfloat16
    if rhs_pre:
        # [n_steps, 8, 128] -> [8, n_steps*128]
        rhs = rhs.transpose(1, 0, 2).reshape(8, n_steps * 128)
    return {
        **extra,
        "wsb": wsb.astype(wdt_np),
        "xw": xw.astype(ml_dtypes.bfloat16),
        "rhs": rhs.astype(ml_dtypes.bfloat16),
        "h0t": h0t.astype(ml_dtypes.bfloat16),
        "c0t": c0t.astype(np.float32),
        "wfc": wfc.astype(ml_dtypes.bfloat16),
        "bfc": b_fc.reshape(OUT, 1).astype(np.float32),
    }


def _env_cfg():
    return dict(
        groups=int(os.environ.get("LSTM_GROUPS", "1")),
        by_gate=bool(int(os.environ.get("LSTM_BY_GATE", "1"))),
        fp8=bool(int(os.environ.get("LSTM_FP8", "1"))),
        rhs_pre=bool(int(os.environ.get("LSTM_RHS_PRE", "1"))),
        tail_k=int(os.environ.get("LSTM_TAIL", "16")),
        do_mm=bool(int(os.environ.get("LSTM_DO_MM", "1"))),
        do_elem=bool(int(os.environ.get("LSTM_DO_ELEM", "1"))),
        chain_split=int(os.environ.get("LSTM_CHAIN_SPLIT", "1")),
        tail_split=int(os.environ.get("LSTM_TAIL_SPLIT", "2")),
        k_outer=int(os.environ.get("LSTM_K_OUTER", "0")),
    )


def kernel(y_hist, h0, c0, W_ih, W_hh, b_ih, b_hh, W_fc, b_fc, **kw):
    n_steps = int(os.environ.get("LSTM_N_STEPS", S))
    cfg = _env_cfg()
    groups = cfg["groups"]
    by_gate = cfg["by_gate"]
    fp8 = cfg["fp8"]
    rhs_pre = cfg["rhs_pre"]
    y_hist = np.asarray(y_hist, np.float32)
    h0 = np.asarray(h0, np.float32)
    c0 = np.asarray(c0, np.float32)
    W_ih = np.asarray(W_ih, np.float32)
    W_hh = np.asarray(W_hh, np.float32)
    bias = np.asarray(b_ih, np.float32) + np.asarray(b_hh, np.float32)
    W_fc = np.asarray(W_fc, np.float32)
    b_fc = np.asarray(b_fc, np.float32)

    key = (n_steps,) + tuple(sorted(cfg.items()))
    if key not in _BUILD_CACHE:
        _BUILD_CACHE[key] = build_kernel(n_steps, **cfg)
    nc = _BUILD_CACHE[key]

    in_maps = [
        _prep_core_inputs(c, y_hist, h0, c0, W_ih, W_hh, bias, W_fc, b_fc, n_steps,
                          by_gate=by_gate, fp8=fp8, rhs_pre=rhs_pre)
        for c in range(NCORES)
    ]
    res = run_bass_kernel_spmd(
        nc,
        in_maps,
        core_ids=list(range(NCORES)),
        trace=bool(int(os.environ.get("LSTM_TRACE", "0"))),
    )
    kernel.last_results = res
    out = np.empty((B, OUT), np.float32)
    for c in range(NCORES):
        out[c * BL : (c + 1) * BL, :] = np.asarray(
            res.results[c]["outt"], np.float32
        ).T
    return out



# revision 16
# speedup vs baseline: 1.2713x; 1.2713x over previous
"""Trainium2 Bass kernel for nn_Decoder (LSTM(input=1,hidden=512) over S=256 steps + FC).

Data-parallel over batch: B=256 -> 32 rows/core on 8 cores; weights replicated.
Feature-major layout (gate features on SBUF partitions, batch on the free dim);
the recurrent matmul is weight-stationary: 64 [128,128] lhsT tiles per step,
one PSUM bank per gate (order f,g,i,o), one accumulation group per bank.

Perf design (vs. the ~904us bf16 baseline, LDWEIGHTS-byte-bound at ~54ns/tile):
- fp8(e4m3) weights, scaled x64 on the host (clears the subnormal floor), with
  bf16 moving operand: halves LDWEIGHTS bytes -> ~21ns/tile pairs. The 1/64
  descale rides the gate activations' free `scale` parameter.
- LSTM state error from fp8 weights saturates (forget-gate damping), so the
  LAST `tail_k`=16 steps run against a second bf16 copy of the weights, which
  collapses the final-state error to the bf16 noise floor (~2.4e-3).
- The whole rhs sequence ([y_t;1] blocks for the x*W_ih+bias fold-in) is
  preloaded to SBUF once: zero in-loop DMA.
- tanh(c) is emitted right after the c-update (i-bank phase) so it runs on
  ScalarE under the o-bank matmuls; the exposed per-step tail is only
  sigmoid(o)->h-mul, chunked in halves (tail_split=2) so the next step's
  matmuls start after the first 64 columns of h.
"""

import os
import sys

sys.path.insert(0, "/opt/trn_rl_repo")

import ml_dtypes
import numpy as np

import concourse.mybir as mybir
import concourse.tile as tile
from concourse import bacc
from concourse.bass_utils import run_bass_kernel_spmd

B, S, H, OUT = 256, 256, 512, 128
NCORES = 8
BL = B // NCORES  # 32 batch rows per core

bf16 = mybir.dt.bfloat16
f32 = mybir.dt.float32

# gate slot order within a slice's 128 cols: [i | f | o | g]
GS_TG = [0, 1, 3, 2]  # gate slot -> torch gate row-block index (i,f,g,o order)

_BUILD_CACHE: dict = {}


def _feat_index():
    """feat[a, gs, p] = row index in W_hh/b for slice a, gate slot gs, partition p."""
    a = np.arange(4)[:, None, None]
    gs = np.array(GS_TG)[None, :, None]
    p = np.arange(128)[None, None, :]
    return (gs * H + a * 128 + p).astype(np.int64)  # (4,4,128)


def step_body(nc, n_steps, Hbuf, C, W, XW, rhsp, work, psum, rhsd, groups,
              do_mm=True, do_elem=True, split_h=False, split_sig=False):
    AF = mybir.ActivationFunctionType
    A = 4 // groups  # hidden slices per group
    ng = 128 * A  # psum cols per group
    kg = 8 * A  # xb-matmul contraction rows per group
    for t in range(n_steps):
        hprev = Hbuf[t % 2]
        hnext = Hbuf[(t + 1) % 2]
        rts = [
            rhsp.tile([kg, ng], bf16, name=f"rt{g}", tag=f"rt{g}")
            for g in range(groups)
        ]
        for g in range(groups):
            nc.sync.dma_start(
                rts[g][:],
                rhsd[t, g * kg : (g + 1) * kg, g * ng : (g + 1) * ng],
            )
        pbs = [
            psum.tile([128, ng], f32, name=f"pb{g}", tag=f"pb{g}")
            for g in range(groups)
        ]
        for g in range(groups):
            pb = pbs[g]
            # xb matmul first: writes the whole group tile (start=True), so
            # every later matmul accumulates onto set has_written bits and
            # emission order of the disjoint gate regions can't corrupt data.
            nc.tensor.matmul(
                pb[:],
                XW[g][:],
                rts[g][:],
                start=True,
                stop=not do_mm,
                skip_group_check=True,
            )
            # gate matmuls: g-region first so tanh(g) can run under the rest
            if do_mm:
                for gs in (3, 0, 1, 2):
                    for al in range(A):
                        a = g * A + al
                        for k in range(4):
                            w_off = ((a * 4 + gs) * 4 + k) * 128
                            nc.tensor.matmul(
                                pb[:, al * 128 + 32 * gs : al * 128 + 32 * gs + 32],
                                W[:, w_off : w_off + 128],
                                hprev[:, 32 * k : 32 * k + 32],
                                start=False,
                                stop=(gs == 2 and al == A - 1 and k == 3),
                                skip_group_check=True,
                            )
        if not do_elem:
            continue
        for g in range(groups):
            pb3 = pbs[g].rearrange("p (a c) -> p a c", c=128)  # (128, A, 128)
            cg3 = C[:, g * 32 * A : (g + 1) * 32 * A].rearrange(
                "p (a c) -> p a c", c=32
            )  # (128, A, 32)
            Gt = work.tile([128, A, 32], f32, name="tg", tag="tg")
            nc.scalar.activation(Gt[:], pb3[:, :, 96:128], AF.Tanh)
            if split_sig:
                Sf = work.tile([128, A, 32], f32, name="sf", tag="sf")
                nc.scalar.activation(Sf[:], pb3[:, :, 32:64], AF.Sigmoid)
                Sio = work.tile([128, A, 2, 32], f32, name="sio", tag="sio")
                nc.scalar.activation(
                    Sio[:],
                    pb3.rearrange("p a c -> p a c")[:, :, 0:96].rearrange(
                        "p a (i c) -> p a i c", c=32
                    )[:, :, 0::2, :],
                    AF.Sigmoid,
                )
                s_i, s_f, s_o = Sio[:, :, 0, :], Sf[:], Sio[:, :, 1, :]
            else:
                Sg = work.tile([128, A, 96], f32, name="sig", tag="sig")
                nc.scalar.activation(Sg[:], pb3[:, :, 0:96], AF.Sigmoid)
                s_i, s_f, s_o = Sg[:, :, 0:32], Sg[:, :, 32:64], Sg[:, :, 64:96]
            T2 = work.tile([128, A, 32], f32, name="t2", tag="t2")
            nc.vector.tensor_mul(out=T2[:], in0=s_f, in1=cg3)
            T1 = work.tile([128, A, 32], f32, name="t1", tag="t1")
            nc.vector.tensor_mul(out=T1[:], in0=s_i, in1=Gt[:])
            nc.vector.tensor_add(out=cg3, in0=T1[:], in1=T2[:])
            TC = work.tile([128, A, 32], f32, name="tc", tag="tc")
            nc.scalar.activation(TC[:], cg3, AF.Tanh)
            hout = hnext[:, g * 32 * A : (g + 1) * 32 * A].rearrange(
                "p (a c) -> p a c", c=32
            )
            if split_h:
                for al in range(A):
                    nc.vector.tensor_mul(
                        out=hout[:, al : al + 1, :],
                        in0=s_o[:, al : al + 1, :],
                        in1=TC[:, al : al + 1, :],
                    )
            else:
                nc.vector.tensor_mul(out=hout, in0=s_o, in1=TC[:])


def step_body_v3(nc, n_steps, Hbuf, C, W, XW, rhsp, work, psum, rhsd,
                 do_mm=True, do_elem=True, split_h=False, chain_split=1,
                 tail_split=None, k_outer=False, inv_s=1.0, RT=None,
                 Wtail=None, tail_k=0):
    if tail_split is None:
        tail_split = chain_split
    """Bank-per-gate layout: each gate's 4 hidden-slices accumulate in its own
    PSUM bank (order f,g,i,o), so per-gate activations overlap the next gate's
    matmuls on a different bank.  Only sigmoid(o) -> h remains after the MM
    phase.  xb matmuls are emitted one step ahead so they fill the PE bubble
    during the activation tail.

    inv_s: descale factor applied in the gate activations (weights were
    pre-scaled by 1/inv_s on the host, e.g. for fp8 quantization).
    RT: optional preloaded rhs tile [8, n_steps*128]; skips per-step DMA."""
    AF = mybir.ActivationFunctionType

    def alloc_step(t):
        if RT is not None:
            rt_ap = RT[:, t * 128 : (t + 1) * 128]
        else:
            rt = rhsp.tile([8, 128], bf16, name="rt", tag="rt")
            nc.sync.dma_start(rt[:], rhsd[t])
            rt_ap = rt[:]
        pbs = [
            psum.tile([128, 128], f32, name=f"pb{b}", tag=f"pb{b}")
            for b in range(4)
        ]
        for b in range(4):
            nc.tensor.matmul(
                pbs[b][:], XW[b][:], rt_ap, start=True, stop=False,
                skip_group_check=True,
            )
        return pbs

    def hsl(hbuf, k):
        return hbuf[:, 32 * k : 32 * k + 32]

    pbs_cur = alloc_step(0)
    import os as _os
    fake_h = bool(int(_os.environ.get("LSTM_FAKE_H", "0")))
    for t in range(n_steps):
        Wt = Wtail if (Wtail is not None and t >= n_steps - tail_k) else W
        hprev = Hbuf[0] if fake_h else Hbuf[t % 2]
        hnext = Hbuf[(t + 1) % 2]
        pbs = pbs_cur
        acts = {}
        for b in range(4):  # bank order f, g, i, o
            pb = pbs[b]
            if do_mm:
                if k_outer == 3:  # al-major within k-halves
                    mm_iter = ([(al, k) for al in range(4) for k in (0, 1)]
                               + [(al, k) for al in range(4) for k in (2, 3)])
                elif k_outer == 2:  # k-pair-major: matches 64-col hnext chunks
                    mm_iter = ([(al, k) for k in (0, 1) for al in range(4)]
                               + [(al, k) for k in (2, 3) for al in range(4)])
                elif k_outer:
                    mm_iter = [(al, k) for k in range(4) for al in range(4)]
                else:
                    mm_iter = [(al, k) for al in range(4) for k in range(4)]
                for idx, (al, k) in enumerate(mm_iter):
                    w_off = ((b * 4 + al) * 4 + k) * 128
                    nc.tensor.matmul(
                        pb[:, al * 32 : al * 32 + 32],
                        Wt[:, w_off : w_off + 128],
                        hsl(hprev, k),
                        start=False,
                        stop=(idx == 15),
                        skip_group_check=True,
                    )
            if not do_elem:
                continue
            if b == 0:  # f
                Sf = work.tile([128, 128], f32, name="sf", tag="sf")
                nc.scalar.activation(Sf[:], pb[:], AF.Sigmoid, scale=inv_s)
                T2 = work.tile([128, 128], f32, name="t2", tag="t2")
                import os as _os3
                if bool(int(_os3.environ.get("LSTM_GP_T2", "0"))):
                    nc.gpsimd.tensor_mul(out=T2[:], in0=Sf[:], in1=C[:])
                else:
                    nc.vector.tensor_mul(out=T2[:], in0=Sf[:], in1=C[:])
                acts["T2"] = T2
            elif b == 1:  # g
                Gt = work.tile([128, 128], f32, name="tg", tag="tg")
                nc.scalar.activation(Gt[:], pb[:], AF.Tanh, scale=inv_s)
                acts["G"] = Gt
            elif b == 2:  # i
                cs = chain_split
                w_ = 128 // cs
                Si = work.tile([128, 128], f32, name="si", tag="si")
                T1 = work.tile([128, 128], f32, name="t1", tag="t1")
                for hh in range(cs):
                    sl_ = slice(hh * w_, (hh + 1) * w_)
                    nc.scalar.activation(Si[:, sl_], pb[:, sl_], AF.Sigmoid,
                                         scale=inv_s)
                for hh in range(cs):
                    sl_ = slice(hh * w_, (hh + 1) * w_)
                    nc.vector.tensor_mul(
                        out=T1[:, sl_], in0=Si[:, sl_], in1=acts["G"][:, sl_]
                    )
                    nc.vector.tensor_add(
                        out=C[:, sl_], in0=T1[:, sl_], in1=acts["T2"][:, sl_]
                    )
                # tanh(c) here: C is final, so these ACT ops run while the
                # o-bank matmuls occupy the PE, taking tanh off the tail path
                TC = work.tile([128, 128], f32, name="tc", tag="tc")
                for hh in range(cs):
                    sl_ = slice(hh * w_, (hh + 1) * w_)
                    nc.scalar.activation(TC[:, sl_], C[:, sl_], AF.Tanh)
                acts["TC"] = TC
            else:  # o
                # next step's xb matmuls first: they are dependency-free, so
                # the PE can run them during this step's activation tail
                if t + 1 < n_steps:
                    pbs_cur = alloc_step(t + 1)
                So = work.tile([128, 128], f32, name="so", tag="so")
                TC = acts["TC"]
                import os as _os2
                so_one = bool(int(_os2.environ.get("LSTM_SO_ONE", "0")))
                ts_ = tail_split
                wt = 128 // ts_
                if so_one:
                    # single sigmoid (one ACT access-latency) but keep the
                    # h-multiply chunked so the next step starts early
                    nc.scalar.activation(So[:], pb[:], AF.Sigmoid,
                                         scale=inv_s)
                    for hh in range(ts_):
                        sl_ = slice(hh * wt, (hh + 1) * wt)
                        nc.vector.tensor_mul(
                            out=hnext[:, sl_], in0=So[:, sl_], in1=TC[:, sl_]
                        )
                else:
                    for hh in range(ts_):
                        sl_ = slice(hh * wt, (hh + 1) * wt)
                        nc.scalar.activation(So[:, sl_], pb[:, sl_],
                                             AF.Sigmoid, scale=inv_s)
                        nc.vector.tensor_mul(
                            out=hnext[:, sl_], in0=So[:, sl_], in1=TC[:, sl_]
                        )
        if not do_elem and t + 1 < n_steps:
            pbs_cur = alloc_step(t + 1)


def build_kernel(n_steps: int, outer: int = 1, groups: int = 1,
                 do_mm=True, do_elem=True, split_h=False, split_sig=False,
                 by_gate=False, chain_split=1, tail_split=None, k_outer=False,
                 fp8=False, rhs_pre=False, scale=64.0, tail_k=16):
    """outer>1 wraps the recurrence in a device-side repeat loop (bench only)."""
    wdt = mybir.dt.float8e4 if fp8 else bf16
    nc = bacc.Bacc(None)
    wsb = nc.declare_dram_parameter("wsb", [128, 64 * 128], wdt, isOutput=False)
    if fp8 and tail_k:
        wsb2 = nc.declare_dram_parameter("wsb2", [128, 64 * 128], bf16,
                                         isOutput=False)
    xw = nc.declare_dram_parameter("xw", [32, 128], bf16, isOutput=False)
    if rhs_pre:
        assert by_gate
        rhs_shape = [8, n_steps * 128]
    else:
        rhs_shape = [n_steps, 8, 128] if by_gate else [n_steps, 32, 512]
    rhsd = nc.declare_dram_parameter("rhs", rhs_shape, bf16, isOutput=False)
    h0t = nc.declare_dram_parameter("h0t", [128, 128], bf16, isOutput=False)
    c0t = nc.declare_dram_parameter("c0t", [128, 128], f32, isOutput=False)
    wfc = nc.declare_dram_parameter("wfc", [128, 512], bf16, isOutput=False)
    bfc = nc.declare_dram_parameter("bfc", [128, 1], f32, isOutput=False)
    outt = nc.declare_dram_parameter("outt", [128, BL], f32, isOutput=True)

    with tile.TileContext(nc) as tc:
        with (
            tc.tile_pool(name="const", bufs=1) as constp,
            tc.tile_pool(name="rhsp", bufs=6) as rhsp,
            tc.tile_pool(name="work", bufs=3) as work,
            tc.tile_pool(name="psum", bufs=2, space="PSUM") as psum,
        ):
            W = constp.tile([128, 64 * 128], wdt, tag="W")
            nc.sync.dma_start(W[:], wsb[:])
            Wtail = None
            if fp8 and tail_k:
                Wtail = constp.tile([128, 64 * 128], bf16, tag="Wtail")
            kg_ = 8 if by_gate else 8 * (4 // groups)
            n_xw = 4 if by_gate else groups
            XW = [
                constp.tile([kg_, 128], bf16, name=f"XW{g}", tag=f"XW{g}")
                for g in range(n_xw)
            ]
            RT = None
            if rhs_pre:
                RT = constp.tile([8, n_steps * 128], bf16, tag="RT")
                nc.sync.dma_start(RT[:], rhsd[:])
            for g in range(n_xw):
                nc.sync.dma_start(XW[g][:], xw[g * kg_ : (g + 1) * kg_, :])
            WF = constp.tile([128, 512], bf16, tag="WF")
            nc.sync.dma_start(WF[:], wfc[:])
            BF = constp.tile([128, 1], f32, tag="BF")
            nc.sync.dma_start(BF[:], bfc[:])
            C = constp.tile([128, 128], f32, tag="C")
            nc.sync.dma_start(C[:], c0t[:])
            Hbuf = [constp.tile([128, 128], bf16, name=f"H{p}", tag=f"H{p}")
                    for p in range(2)]
            for p in range(1 if do_elem else 2):
                nc.sync.dma_start(Hbuf[p][:], h0t[:])
            if Wtail is not None:
                # queued last: not needed until step n_steps - tail_k, so the
                # first steps only wait on the fp8 weights + rhs + h0/c0
                nc.sync.dma_start(Wtail[:], wsb2[:])

            import contextlib

            loop_cm = tc.For_i(0, outer, 1) if outer > 1 else contextlib.nullcontext()
            with loop_cm:
                if by_gate:
                    step_body_v3(nc, n_steps, Hbuf, C, W, XW, rhsp, work, psum,
                                 rhsd, do_mm=do_mm, do_elem=do_elem,
                                 split_h=split_h, chain_split=chain_split,
                                 tail_split=tail_split, k_outer=k_outer,
                                 inv_s=(1.0 / scale) if fp8 else 1.0, RT=RT,
                                 Wtail=Wtail, tail_k=tail_k)
                else:
                    step_body(nc, n_steps, Hbuf, C, W, XW, rhsp, work, psum,
                              rhsd, groups, do_mm=do_mm, do_elem=do_elem,
                              split_h=split_h, split_sig=split_sig)

            # final FC: outT(128, 32) = W_fc @ h_final (+ b_fc)
            hfin = Hbuf[n_steps % 2]
            pfc = psum.tile([128, 128], f32, name="pfc", tag="pb0")
            for k in range(4):
                nc.tensor.matmul(
                    pfc[:, 0:32],
                    WF[:, 128 * k : 128 * k + 128],
                    hfin[:, 32 * k : 32 * k + 32],
                    start=(k == 0),
                    stop=(k == 3),
                )
            osb = work.tile([128, BL], f32, name="osb", tag="osb")
            nc.vector.tensor_scalar_add(osb[:], pfc[:, 0:32], BF[:])
            nc.sync.dma_start(outt[:], osb[:])

    nc.finalize()
    return nc


def _prep_core_inputs(core, y_hist, h0, c0, W_ih, W_hh, bias, W_fc, b_fc, n_steps,
                      by_gate=False, fp8=False, rhs_pre=False, scale=64.0):
    sl = slice(core * BL, (core + 1) * BL)
    y_c = y_hist[sl]  # (32, S)

    if by_gate:
        # feat[b, al, p]: bank order f,g,i,o over torch row-blocks i,f,g,o
        bg = np.array([1, 2, 0, 3])[:, None, None]
        al = np.arange(4)[None, :, None]
        p = np.arange(128)[None, None, :]
        feat = (bg * H + al * 128 + p).astype(np.int64)  # (4,4,128)
        # wsb[r, ((b*4+al)*4+k)*128 + m] = W_hh[feat[b,al,m], k*128+r]
        t = W_hh[feat].reshape(4, 4, 128, 4, 128)  # (b,al,m,k,r)
        wsb = np.ascontiguousarray(t.transpose(4, 0, 1, 3, 2).reshape(128, 64 * 128))
        # xw[b*8+al*2+j, p]
        wih_f = W_ih[:, 0][feat]
        b_f = bias[feat]
        xw = np.stack([wih_f, b_f], axis=2).reshape(32, 128)
        # rhs[t, al*2+j, al*32+n]
        rhs = np.zeros((n_steps, 8, 128), np.float32)
        for a in range(4):
            cols = slice(a * 32, a * 32 + 32)
            rhs[:, a * 2 + 0, cols] = y_c.T[:n_steps]
            rhs[:, a * 2 + 1, cols] = 1.0
    else:
        feat = _feat_index()  # (4,4,128)
        # wsb[r, ((a*4+gs)*4+k)*128 + m] = W_hh[feat[a,gs,m], k*128+r]
        t = W_hh[feat]  # (4,4,128m,512)
        t = t.reshape(4, 4, 128, 4, 128)  # (a,gs,m,k,r)
        wsb = np.ascontiguousarray(t.transpose(4, 0, 1, 3, 2).reshape(128, 64 * 128))
        # xw[a*8+s*2+j, p] = W_ih[feat[a,s,p]] (j=0) or bias[feat[a,s,p]] (j=1)
        wih_f = W_ih[:, 0][feat]  # (4,4,128)
        b_f = bias[feat]  # (4,4,128)
        xw = np.stack([wih_f, b_f], axis=2).reshape(32, 128)  # (a,s,j,p)->(32,128)
        # rhs[t, a*8+s*2+j, a*128+s*32+n] = y[n,t] (j=0) or 1.0 (j=1)
        rhs = np.zeros((n_steps, 32, 512), np.float32)
        for a in range(4):
            for s in range(4):
                cols = slice(a * 128 + s * 32, a * 128 + s * 32 + 32)
                rhs[:, a * 8 + s * 2 + 0, cols] = y_c.T[:n_steps]
                rhs[:, a * 8 + s * 2 + 1, cols] = 1.0

    def t128(x):  # (32, 512) -> (128, 128): out[p, 32k+n] = x[n, k*128+p]
        return np.ascontiguousarray(
            x.T.reshape(4, 128, BL).transpose(1, 0, 2).reshape(128, 4 * BL)
        )

    h0t = t128(h0[sl])
    c0t = t128(c0[sl])

    # wfc[p, k*128+m] = W_fc[m, k*128+p]
    wfc = np.ascontiguousarray(
        W_fc.reshape(OUT, 4, 128).transpose(2, 1, 0).reshape(128, 512)
    )

    extra = {}
    if fp8:
        wdt_np = ml_dtypes.float8_e4m3
        wsb = wsb * scale
        xw = xw * scale
        extra["wsb2"] = wsb.astype(ml_dtypes.bfloat16)
    else:
        wdt_np = ml_dtypes.bfloat16
    if rhs_pre:
        # [n_steps, 8, 128] -> [8, n_steps*128]
        rhs = rhs.transpose(1, 0, 2).reshape(8, n_steps * 128)
    return {
        **extra,
        "wsb": wsb.astype(wdt_np),
        "xw": xw.astype(ml_dtypes.bfloat16),
        "rhs": rhs.astype(ml_dtypes.bfloat16),
        "h0t": h0t.astype(ml_dtypes.bfloat16),
        "c0t": c0t.astype(np.float32),
        "wfc": wfc.astype(ml_dtypes.bfloat16),
        "bfc": b_fc.reshape(OUT, 1).astype(np.float32),
    }


def _env_cfg():
    return dict(
        groups=int(os.environ.get("LSTM_GROUPS", "1")),
        by_gate=bool(int(os.environ.get("LSTM_BY_GATE", "1"))),
        fp8=bool(int(os.environ.get("LSTM_FP8", "1"))),
        rhs_pre=bool(int(os.environ.get("LSTM_RHS_PRE", "1"))),
        tail_k=int(os.environ.get("LSTM_TAIL", "8")),
        do_mm=bool(int(os.environ.get("LSTM_DO_MM", "1"))),
        do_elem=bool(int(os.environ.get("LSTM_DO_ELEM", "1"))),
        chain_split=int(os.environ.get("LSTM_CHAIN_SPLIT", "1")),
        tail_split=int(os.environ.get("LSTM_TAIL_SPLIT", "2")),
        k_outer=int(os.environ.get("LSTM_K_OUTER", "0")),
    )


def kernel(y_hist, h0, c0, W_ih, W_hh, b_ih, b_hh, W_fc, b_fc, **kw):
    n_steps = int(os.environ.get("LSTM_N_STEPS", S))
    cfg = _env_cfg()
    groups = cfg["groups"]
    by_gate = cfg["by_gate"]
    fp8 = cfg["fp8"]
    rhs_pre = cfg["rhs_pre"]
    y_hist = np.asarray(y_hist, np.float32)
    h0 = np.asarray(h0, np.float32)
    c0 = np.asarray(c0, np.float32)
    W_ih = np.asarray(W_ih, np.float32)
    W_hh = np.asarray(W_hh, np.float32)
    bias = np.asarray(b_ih, np.float32) + np.asarray(b_hh, np.float32)
    W_fc = np.asarray(W_fc, np.float32)
    b_fc = np.asarray(b_fc, np.float32)

    key = (n_steps,) + tuple(sorted(cfg.items()))
    if key not in _BUILD_CACHE:
        _BUILD_CACHE[key] = build_kernel(n_steps, **cfg)
    nc = _BUILD_CACHE[key]

    in_maps = [
        _prep_core_inputs(c, y_hist, h0, c0, W_ih, W_hh, bias, W_fc, b_fc, n_steps,
                          by_gate=by_gate, fp8=fp8, rhs_pre=rhs_pre)
        for c in range(NCORES)
    ]
    res = run_bass_kernel_spmd(
        nc,
        in_maps,
        core_ids=list(range(NCORES)),
        trace=bool(int(os.environ.get("LSTM_TRACE", "0"))),
    )
    kernel.last_results = res
    out = np.empty((B, OUT), np.float32)
    for c in range(NCORES):
        out[c * BL : (c + 1) * BL, :] = np.asarray(
            res.results[c]["outt"], np.float32
        ).T
    return out



# revision 17
# speedup vs baseline: 2.1727x; 1.7090x over previous
"""Trainium2 Bass kernel for nn_Decoder (LSTM(input=1,hidden=512) over S=256 steps + FC).

Data-parallel over batch: B=256 -> 32 rows/core on 8 cores; weights replicated.
Feature-major layout (gate features on SBUF partitions, batch on the free dim);
the recurrent matmul is weight-stationary: 64 [128,128] lhsT tiles per step,
one PSUM bank per gate (order f,g,i,o), one accumulation group per bank.

Perf design (vs. the ~904us bf16 baseline, LDWEIGHTS-byte-bound at ~54ns/tile):
- fp8(e4m3) weights, scaled x64 on the host (clears the subnormal floor), with
  bf16 moving operand: halves LDWEIGHTS bytes -> ~21ns/tile pairs. The 1/64
  descale rides the gate activations' free `scale` parameter.
- LSTM state error from fp8 weights saturates (forget-gate damping), so the
  LAST `tail_k`=8 steps run against a second bf16 copy of the weights, which
  collapses the final-state error to the bf16 noise floor (~2.4e-3).
- The whole rhs sequence ([y_t;1] blocks for the x*W_ih+bias fold-in) is
  preloaded to SBUF once: zero in-loop DMA.
- tanh(c) is emitted right after the c-update (i-bank phase) so it runs on
  ScalarE under the o-bank matmuls; the exposed per-step tail is only
  sigmoid(o)->h-mul, chunked in halves (tail_split=2) so the next step's
  matmuls start after the first 64 columns of h.
"""

import os
import sys

sys.path.insert(0, "/opt/trn_rl_repo")

import ml_dtypes
import numpy as np

import concourse.mybir as mybir
import concourse.tile as tile
from concourse import bacc
from concourse.bass_utils import run_bass_kernel_spmd

B, S, H, OUT = 256, 256, 512, 128
NCORES = 8
BL = B // NCORES  # 32 batch rows per core

bf16 = mybir.dt.bfloat16
f32 = mybir.dt.float32

# gate slot order within a slice's 128 cols: [i | f | o | g]
GS_TG = [0, 1, 3, 2]  # gate slot -> torch gate row-block index (i,f,g,o order)

_BUILD_CACHE: dict = {}


def _feat_index():
    """feat[a, gs, p] = row index in W_hh/b for slice a, gate slot gs, partition p."""
    a = np.arange(4)[:, None, None]
    gs = np.array(GS_TG)[None, :, None]
    p = np.arange(128)[None, None, :]
    return (gs * H + a * 128 + p).astype(np.int64)  # (4,4,128)


def step_body(nc, n_steps, Hbuf, C, W, XW, rhsp, work, psum, rhsd, groups,
              do_mm=True, do_elem=True, split_h=False, split_sig=False):
    AF = mybir.ActivationFunctionType
    A = 4 // groups  # hidden slices per group
    ng = 128 * A  # psum cols per group
    kg = 8 * A  # xb-matmul contraction rows per group
    for t in range(n_steps):
        hprev = Hbuf[t % 2]
        hnext = Hbuf[(t + 1) % 2]
        rts = [
            rhsp.tile([kg, ng], bf16, name=f"rt{g}", tag=f"rt{g}")
            for g in range(groups)
        ]
        for g in range(groups):
            nc.sync.dma_start(
                rts[g][:],
                rhsd[t, g * kg : (g + 1) * kg, g * ng : (g + 1) * ng],
            )
        pbs = [
            psum.tile([128, ng], f32, name=f"pb{g}", tag=f"pb{g}")
            for g in range(groups)
        ]
        for g in range(groups):
            pb = pbs[g]
            # xb matmul first: writes the whole group tile (start=True), so
            # every later matmul accumulates onto set has_written bits and
            # emission order of the disjoint gate regions can't corrupt data.
            nc.tensor.matmul(
                pb[:],
                XW[g][:],
                rts[g][:],
                start=True,
                stop=not do_mm,
                skip_group_check=True,
            )
            # gate matmuls: g-region first so tanh(g) can run under the rest
            if do_mm:
                for gs in (3, 0, 1, 2):
                    for al in range(A):
                        a = g * A + al
                        for k in range(4):
                            w_off = ((a * 4 + gs) * 4 + k) * 128
                            nc.tensor.matmul(
                                pb[:, al * 128 + 32 * gs : al * 128 + 32 * gs + 32],
                                W[:, w_off : w_off + 128],
                                hprev[:, 32 * k : 32 * k + 32],
                                start=False,
                                stop=(gs == 2 and al == A - 1 and k == 3),
                                skip_group_check=True,
                            )
        if not do_elem:
            continue
        for g in range(groups):
            pb3 = pbs[g].rearrange("p (a c) -> p a c", c=128)  # (128, A, 128)
            cg3 = C[:, g * 32 * A : (g + 1) * 32 * A].rearrange(
                "p (a c) -> p a c", c=32
            )  # (128, A, 32)
            Gt = work.tile([128, A, 32], f32, name="tg", tag="tg")
            nc.scalar.activation(Gt[:], pb3[:, :, 96:128], AF.Tanh)
            if split_sig:
                Sf = work.tile([128, A, 32], f32, name="sf", tag="sf")
                nc.scalar.activation(Sf[:], pb3[:, :, 32:64], AF.Sigmoid)
                Sio = work.tile([128, A, 2, 32], f32, name="sio", tag="sio")
                nc.scalar.activation(
                    Sio[:],
                    pb3.rearrange("p a c -> p a c")[:, :, 0:96].rearrange(
                        "p a (i c) -> p a i c", c=32
                    )[:, :, 0::2, :],
                    AF.Sigmoid,
                )
                s_i, s_f, s_o = Sio[:, :, 0, :], Sf[:], Sio[:, :, 1, :]
            else:
                Sg = work.tile([128, A, 96], f32, name="sig", tag="sig")
                nc.scalar.activation(Sg[:], pb3[:, :, 0:96], AF.Sigmoid)
                s_i, s_f, s_o = Sg[:, :, 0:32], Sg[:, :, 32:64], Sg[:, :, 64:96]
            T2 = work.tile([128, A, 32], f32, name="t2", tag="t2")
            nc.vector.tensor_mul(out=T2[:], in0=s_f, in1=cg3)
            T1 = work.tile([128, A, 32], f32, name="t1", tag="t1")
            nc.vector.tensor_mul(out=T1[:], in0=s_i, in1=Gt[:])
            nc.vector.tensor_add(out=cg3, in0=T1[:], in1=T2[:])
            TC = work.tile([128, A, 32], f32, name="tc", tag="tc")
            nc.scalar.activation(TC[:], cg3, AF.Tanh)
            hout = hnext[:, g * 32 * A : (g + 1) * 32 * A].rearrange(
                "p (a c) -> p a c", c=32
            )
            if split_h:
                for al in range(A):
                    nc.vector.tensor_mul(
                        out=hout[:, al : al + 1, :],
                        in0=s_o[:, al : al + 1, :],
                        in1=TC[:, al : al + 1, :],
                    )
            else:
                nc.vector.tensor_mul(out=hout, in0=s_o, in1=TC[:])


def step_body_v3(nc, n_steps, Hbuf, C, W, XW, rhsp, work, psum, rhsd,
                 do_mm=True, do_elem=True, split_h=False, chain_split=1,
                 tail_split=None, k_outer=False, inv_s=1.0, RT=None,
                 Wtail=None, tail_k=0):
    if tail_split is None:
        tail_split = chain_split
    """Bank-per-gate layout: each gate's 4 hidden-slices accumulate in its own
    PSUM bank (order f,g,i,o), so per-gate activations overlap the next gate's
    matmuls on a different bank.  Only sigmoid(o) -> h remains after the MM
    phase.  xb matmuls are emitted one step ahead so they fill the PE bubble
    during the activation tail.

    inv_s: descale factor applied in the gate activations (weights were
    pre-scaled by 1/inv_s on the host, e.g. for fp8 quantization).
    RT: optional preloaded rhs tile [8, n_steps*128]; skips per-step DMA."""
    AF = mybir.ActivationFunctionType

    def alloc_step(t):
        if RT is not None:
            rt_ap = RT[:, t * 128 : (t + 1) * 128]
        else:
            rt = rhsp.tile([8, 128], bf16, name="rt", tag="rt")
            nc.sync.dma_start(rt[:], rhsd[t])
            rt_ap = rt[:]
        pbs = [
            psum.tile([128, 128], f32, name=f"pb{b}", tag=f"pb{b}")
            for b in range(4)
        ]
        for b in range(4):
            nc.tensor.matmul(
                pbs[b][:], XW[b][:], rt_ap, start=True, stop=False,
                skip_group_check=True,
            )
        return pbs

    def hsl(hbuf, k):
        return hbuf[:, 32 * k : 32 * k + 32]

    pbs_cur = alloc_step(0)
    import os as _os
    fake_h = bool(int(_os.environ.get("LSTM_FAKE_H", "0")))
    for t in range(n_steps):
        Wt = Wtail if (Wtail is not None and t >= n_steps - tail_k) else W
        hprev = Hbuf[0] if fake_h else Hbuf[t % 2]
        hnext = Hbuf[(t + 1) % 2]
        pbs = pbs_cur
        acts = {}
        for b in range(4):  # bank order f, g, i, o
            pb = pbs[b]
            if do_mm:
                if k_outer == 3:  # al-major within k-halves
                    mm_iter = ([(al, k) for al in range(4) for k in (0, 1)]
                               + [(al, k) for al in range(4) for k in (2, 3)])
                elif k_outer == 2:  # k-pair-major: matches 64-col hnext chunks
                    mm_iter = ([(al, k) for k in (0, 1) for al in range(4)]
                               + [(al, k) for k in (2, 3) for al in range(4)])
                elif k_outer:
                    mm_iter = [(al, k) for k in range(4) for al in range(4)]
                else:
                    mm_iter = [(al, k) for al in range(4) for k in range(4)]
                for idx, (al, k) in enumerate(mm_iter):
                    w_off = ((b * 4 + al) * 4 + k) * 128
                    nc.tensor.matmul(
                        pb[:, al * 32 : al * 32 + 32],
                        Wt[:, w_off : w_off + 128],
                        hsl(hprev, k),
                        start=False,
                        stop=(idx == 15),
                        skip_group_check=True,
                    )
            if not do_elem:
                continue
            if b == 0:  # f
                Sf = work.tile([128, 128], f32, name="sf", tag="sf")
                nc.scalar.activation(Sf[:], pb[:], AF.Sigmoid, scale=inv_s)
                T2 = work.tile([128, 128], f32, name="t2", tag="t2")
                import os as _os3
                if bool(int(_os3.environ.get("LSTM_GP_T2", "0"))):
                    nc.gpsimd.tensor_mul(out=T2[:], in0=Sf[:], in1=C[:])
                else:
                    nc.vector.tensor_mul(out=T2[:], in0=Sf[:], in1=C[:])
                acts["T2"] = T2
            elif b == 1:  # g
                Gt = work.tile([128, 128], f32, name="tg", tag="tg")
                nc.scalar.activation(Gt[:], pb[:], AF.Tanh, scale=inv_s)
                acts["G"] = Gt
            elif b == 2:  # i
                cs = chain_split
                w_ = 128 // cs
                Si = work.tile([128, 128], f32, name="si", tag="si")
                T1 = work.tile([128, 128], f32, name="t1", tag="t1")
                for hh in range(cs):
                    sl_ = slice(hh * w_, (hh + 1) * w_)
                    nc.scalar.activation(Si[:, sl_], pb[:, sl_], AF.Sigmoid,
                                         scale=inv_s)
                for hh in range(cs):
                    sl_ = slice(hh * w_, (hh + 1) * w_)
                    nc.vector.tensor_mul(
                        out=T1[:, sl_], in0=Si[:, sl_], in1=acts["G"][:, sl_]
                    )
                    nc.vector.tensor_add(
                        out=C[:, sl_], in0=T1[:, sl_], in1=acts["T2"][:, sl_]
                    )
                # tanh(c) here: C is final, so these ACT ops run while the
                # o-bank matmuls occupy the PE, taking tanh off the tail path
                TC = work.tile([128, 128], f32, name="tc", tag="tc")
                for hh in range(cs):
                    sl_ = slice(hh * w_, (hh + 1) * w_)
                    nc.scalar.activation(TC[:, sl_], C[:, sl_], AF.Tanh)
                acts["TC"] = TC
            else:  # o
                # next step's xb matmuls first: they are dependency-free, so
                # the PE can run them during this step's activation tail
                if t + 1 < n_steps:
                    pbs_cur = alloc_step(t + 1)
                So = work.tile([128, 128], f32, name="so", tag="so")
                TC = acts["TC"]
                import os as _os2
                so_one = bool(int(_os2.environ.get("LSTM_SO_ONE", "0")))
                ts_ = tail_split
                wt = 128 // ts_
                if so_one:
                    # single sigmoid (one ACT access-latency) but keep the
                    # h-multiply chunked so the next step starts early
                    nc.scalar.activation(So[:], pb[:], AF.Sigmoid,
                                         scale=inv_s)
                    for hh in range(ts_):
                        sl_ = slice(hh * wt, (hh + 1) * wt)
                        nc.vector.tensor_mul(
                            out=hnext[:, sl_], in0=So[:, sl_], in1=TC[:, sl_]
                        )
                else:
                    for hh in range(ts_):
                        sl_ = slice(hh * wt, (hh + 1) * wt)
                        nc.scalar.activation(So[:, sl_], pb[:, sl_],
                                             AF.Sigmoid, scale=inv_s)
                        nc.vector.tensor_mul(
                            out=hnext[:, sl_], in0=So[:, sl_], in1=TC[:, sl_]
                        )
        if not do_elem and t + 1 < n_steps:
            pbs_cur = alloc_step(t + 1)


def build_kernel(n_steps: int, outer: int = 1, groups: int = 1,
                 do_mm=True, do_elem=True, split_h=False, split_sig=False,
                 by_gate=False, chain_split=1, tail_split=None, k_outer=False,
                 fp8=False, rhs_pre=False, scale=64.0, tail_k=16):
    """outer>1 wraps the recurrence in a device-side repeat loop (bench only)."""
    wdt = mybir.dt.float8e4 if fp8 else bf16
    nc = bacc.Bacc(None)
    wsb = nc.declare_dram_parameter("wsb", [128, 64 * 128], wdt, isOutput=False)
    if fp8 and tail_k:
        wsb2 = nc.declare_dram_parameter("wsb2", [128, 64 * 128], bf16,
                                         isOutput=False)
    xw = nc.declare_dram_parameter("xw", [32, 128], bf16, isOutput=False)
    if rhs_pre:
        assert by_gate
        rhs_shape = [8, n_steps * 128]
    else:
        rhs_shape = [n_steps, 8, 128] if by_gate else [n_steps, 32, 512]
    rhsd = nc.declare_dram_parameter("rhs", rhs_shape, bf16, isOutput=False)
    h0t = nc.declare_dram_parameter("h0t", [128, 128], bf16, isOutput=False)
    c0t = nc.declare_dram_parameter("c0t", [128, 128], f32, isOutput=False)
    wfc = nc.declare_dram_parameter("wfc", [128, 512], bf16, isOutput=False)
    bfc = nc.declare_dram_parameter("bfc", [128, 1], f32, isOutput=False)
    outt = nc.declare_dram_parameter("outt", [128, BL], f32, isOutput=True)

    with tile.TileContext(nc) as tc:
        with (
            tc.tile_pool(name="const", bufs=1) as constp,
            tc.tile_pool(name="rhsp", bufs=6) as rhsp,
            tc.tile_pool(name="work", bufs=3) as work,
            tc.tile_pool(name="psum", bufs=2, space="PSUM") as psum,
        ):
            W = constp.tile([128, 64 * 128], wdt, tag="W")
            nc.sync.dma_start(W[:], wsb[:])
            Wtail = None
            if fp8 and tail_k:
                Wtail = constp.tile([128, 64 * 128], bf16, tag="Wtail")
            kg_ = 8 if by_gate else 8 * (4 // groups)
            n_xw = 4 if by_gate else groups
            XW = [
                constp.tile([kg_, 128], bf16, name=f"XW{g}", tag=f"XW{g}")
                for g in range(n_xw)
            ]
            RT = None
            if rhs_pre:
                RT = constp.tile([8, n_steps * 128], bf16, tag="RT")
                nc.sync.dma_start(RT[:], rhsd[:])
            for g in range(n_xw):
                nc.sync.dma_start(XW[g][:], xw[g * kg_ : (g + 1) * kg_, :])
            WF = constp.tile([128, 512], bf16, tag="WF")
            nc.sync.dma_start(WF[:], wfc[:])
            BF = constp.tile([128, 1], f32, tag="BF")
            nc.sync.dma_start(BF[:], bfc[:])
            C = constp.tile([128, 128], f32, tag="C")
            nc.sync.dma_start(C[:], c0t[:])
            Hbuf = [constp.tile([128, 128], bf16, name=f"H{p}", tag=f"H{p}")
                    for p in range(2)]
            for p in range(1 if do_elem else 2):
                nc.sync.dma_start(Hbuf[p][:], h0t[:])
            if Wtail is not None:
                # queued last: not needed until step n_steps - tail_k, so the
                # first steps only wait on the fp8 weights + rhs + h0/c0
                nc.sync.dma_start(Wtail[:], wsb2[:])

            import contextlib

            loop_cm = tc.For_i(0, outer, 1) if outer > 1 else contextlib.nullcontext()
            with loop_cm:
                if by_gate:
                    step_body_v3(nc, n_steps, Hbuf, C, W, XW, rhsp, work, psum,
                                 rhsd, do_mm=do_mm, do_elem=do_elem,
                                 split_h=split_h, chain_split=chain_split,
                                 tail_split=tail_split, k_outer=k_outer,
                                 inv_s=(1.0 / scale) if fp8 else 1.0, RT=RT,
                                 Wtail=Wtail, tail_k=tail_k)
                else:
                    step_body(nc, n_steps, Hbuf, C, W, XW, rhsp, work, psum,
                              rhsd, groups, do_mm=do_mm, do_elem=do_elem,
                              split_h=split_h, split_sig=split_sig)

            # final FC: outT(128, 32) = W_fc @ h_final (+ b_fc)
            hfin = Hbuf[n_steps % 2]
            pfc = psum.tile([128, 128], f32, name="pfc", tag="pb0")
            for k in range(4):
                nc.tensor.matmul(
                    pfc[:, 0:32],
                    WF[:, 128 * k : 128 * k + 128],
                    hfin[:, 32 * k : 32 * k + 32],
                    start=(k == 0),
                    stop=(k == 3),
                )
            osb = work.tile([128, BL], f32, name="osb", tag="osb")
            nc.vector.tensor_scalar_add(osb[:], pfc[:, 0:32], BF[:])
            nc.sync.dma_start(outt[:], osb[:])

    nc.finalize()
    return nc


def _prep_core_inputs(core, y_hist, h0, c0, W_ih, W_hh, bias, W_fc, b_fc, n_steps,
                      by_gate=False, fp8=False, rhs_pre=False, scale=64.0):
    sl = slice(core * BL, (core + 1) * BL)
    y_c = y_hist[sl]  # (32, S)

    if by_gate:
        # feat[b, al, p]: bank order f,g,i,o over torch row-blocks i,f,g,o
        bg = np.array([1, 2, 0, 3])[:, None, None]
        al = np.arange(4)[None, :, None]
        p = np.arange(128)[None, None, :]
        feat = (bg * H + al * 128 + p).astype(np.int64)  # (4,4,128)
        # wsb[r, ((b*4+al)*4+k)*128 + m] = W_hh[feat[b,al,m], k*128+r]
        t = W_hh[feat].reshape(4, 4, 128, 4, 128)  # (b,al,m,k,r)
        wsb = np.ascontiguousarray(t.transpose(4, 0, 1, 3, 2).reshape(128, 64 * 128))
        # xw[b*8+al*2+j, p]
        wih_f = W_ih[:, 0][feat]
        b_f = bias[feat]
        xw = np.stack([wih_f, b_f], axis=2).reshape(32, 128)
        # rhs[t, al*2+j, al*32+n]
        rhs = np.zeros((n_steps, 8, 128), np.float32)
        for a in range(4):
            cols = slice(a * 32, a * 32 + 32)
            rhs[:, a * 2 + 0, cols] = y_c.T[:n_steps]
            rhs[:, a * 2 + 1, cols] = 1.0
    else:
        feat = _feat_index()  # (4,4,128)
        # wsb[r, ((a*4+gs)*4+k)*128 + m] = W_hh[feat[a,gs,m], k*128+r]
        t = W_hh[feat]  # (4,4,128m,512)
        t = t.reshape(4, 4, 128, 4, 128)  # (a,gs,m,k,r)
        wsb = np.ascontiguousarray(t.transpose(4, 0, 1, 3, 2).reshape(128, 64 * 128))
        # xw[a*8+s*2+j, p] = W_ih[feat[a,s,p]] (j=0) or bias[feat[a,s,p]] (j=1)
        wih_f = W_ih[:, 0][feat]  # (4,4,128)
        b_f = bias[feat]  # (4,4,128)
        xw = np.stack([wih_f, b_f], axis=2).reshape(32, 128)  # (a,s,j,p)->(32,128)
        # rhs[t, a*8+s*2+j, a*128+s*32+n] = y[n,t] (j=0) or 1.0 (j=1)
        rhs = np.zeros((n_steps, 32, 512), np.float32)
        for a in range(4):
            for s in range(4):
                cols = slice(a * 128 + s * 32, a * 128 + s * 32 + 32)
                rhs[:, a * 8 + s * 2 + 0, cols] = y_c.T[:n_steps]
                rhs[:, a * 8 + s * 2 + 1, cols] = 1.0

    def t128(x):  # (32, 512) -> (128, 128): out[p, 32k+n] = x[n, k*128+p]
        return np.ascontiguousarray(
            x.T.reshape(4, 128, BL).transpose(1, 0, 2).reshape(128, 4 * BL)
        )

    h0t = t128(h0[sl])
    c0t = t128(c0[sl])

    # wfc[p, k*128+m] = W_fc[m, k*128+p]
    wfc = np.ascontiguousarray(
        W_fc.reshape(OUT, 4, 128).transpose(2, 1, 0).reshape(128, 512)
    )

    extra = {}
    if fp8:
        wdt_np = ml_dtypes.float8_e4m3
        wsb = wsb * scale
        xw = xw * scale
        extra["wsb2"] = wsb.astype(ml_dtypes.bfloat16)
    else:
        wdt_np = ml_dtypes.bfloat16
    if rhs_pre:
        # [n_steps, 8, 128] -> [8, n_steps*128]
        rhs = rhs.transpose(1, 0, 2).reshape(8, n_steps * 128)
    return {
        **extra,
        "wsb": wsb.astype(wdt_np),
        "xw": xw.astype(ml_dtypes.bfloat16),
        "rhs": rhs.astype(ml_dtypes.bfloat16),
        "h0t": h0t.astype(ml_dtypes.bfloat16),
        "c0t": c0t.astype(np.float32),
        "wfc": wfc.astype(ml_dtypes.bfloat16),
        "bfc": b_fc.reshape(OUT, 1).astype(np.float32),
    }


def _env_cfg():
    return dict(
        groups=int(os.environ.get("LSTM_GROUPS", "1")),
        by_gate=bool(int(os.environ.get("LSTM_BY_GATE", "1"))),
        fp8=bool(int(os.environ.get("LSTM_FP8", "1"))),
        rhs_pre=bool(int(os.environ.get("LSTM_RHS_PRE", "1"))),
        tail_k=int(os.environ.get("LSTM_TAIL", "8")),
        do_mm=bool(int(os.environ.get("LSTM_DO_MM", "1"))),
        do_elem=bool(int(os.environ.get("LSTM_DO_ELEM", "1"))),
        chain_split=int(os.environ.get("LSTM_CHAIN_SPLIT", "1")),
        tail_split=int(os.environ.get("LSTM_TAIL_SPLIT", "2")),
        k_outer=int(os.environ.get("LSTM_K_OUTER", "0")),
    )


def kernel(y_hist, h0, c0, W_ih, W_hh, b_ih, b_hh, W_fc, b_fc, **kw):
    n_steps = int(os.environ.get("LSTM_N_STEPS", S))
    cfg = _env_cfg()
    groups = cfg["groups"]
    by_gate = cfg["by_gate"]
    fp8 = cfg["fp8"]
    rhs_pre = cfg["rhs_pre"]
    y_hist = np.asarray(y_hist, np.float32)
    h0 = np.asarray(h0, np.float32)
    c0 = np.asarray(c0, np.float32)
    W_ih = np.asarray(W_ih, np.float32)
    W_hh = np.asarray(W_hh, np.float32)
    bias = np.asarray(b_ih, np.float32) + np.asarray(b_hh, np.float32)
    W_fc = np.asarray(W_fc, np.float32)
    b_fc = np.asarray(b_fc, np.float32)

    key = (n_steps,) + tuple(sorted(cfg.items()))
    if key not in _BUILD_CACHE:
        _BUILD_CACHE[key] = build_kernel(n_steps, **cfg)
    nc = _BUILD_CACHE[key]

    in_maps = [
        _prep_core_inputs(c, y_hist, h0, c0, W_ih, W_hh, bias, W_fc, b_fc, n_steps,
                          by_gate=by_gate, fp8=fp8, rhs_pre=rhs_pre)
        for c in range(NCORES)
    ]
    res = run_bass_kernel_spmd(
        nc,
        in_maps,
        core_ids=list(range(NCORES)),
        trace=bool(int(os.environ.get("LSTM_TRACE", "0"))),
    )
    kernel.last_results = res
    out = np.empty((B, OUT), np.float32)
    for c in range(NCORES):
        out[c * BL : (c + 1) * BL, :] = np.asarray(
            res.results[c]["outt"], np.float32
        ).T
    return out

